# revision 14
# baseline (speedup 1.0000x reference)
"""Differentiable Canny edge detector on 8 Trainium2 NeuronCores.

Sharding: pure data parallel over batch (32 images -> 4 per core).

Wire format (the axon tunnel is the bottleneck, ~45 MB/s host->device,
~30 MB/s device->host, ~74 ms fixed cost per array):
  The kernel only ever reads the channel-sum of x (the module's first op
  is a channel mean), so the host ships the reduced sufficient statistic:
  gq = round((c0+c1+c2) * 65535/3) as uint16 (16.8 MB instead of 100.7).
  The 1/65535 dequant is folded into the PE conv weights, so gx/gy come
  out in original units and everything downstream runs on device
  unchanged. The f32 PE weights ride along bit-cast into 96 extra u16
  rows appended to each image (one DRAM input tensor total). Outputs are packed to 1.5 bytes/pixel in one u8 tensor
  [B,3,H,W/2]: plane0/plane1 = hi8 = round(hi * 255/4) at even/odd
  columns, plane2 = (d4_even << 4) | d4_odd with d4 = round((lo - hi) *
  15/0.12) (lo - hi <= 0.11452 analytically). Host decodes hi = hi8/SC,
  lo = hi + d4/SCD. Measured end-to-end L2 ~1.0e-2 (threshold 2e-2),
  dominated by hi8 rounding + rare NMS comparison flips from input
  quantization. Device f32->uint conversions are RNE (verified on
  hardware), matching the np.round model.

Per-core pipeline (per image):
  gray  = gq / 65535                          (dequant folded into PE weights)
  gx    = vert5_sym  . horiz5_anti (gray)     (horiz 5-tap on DVE, vert 5-tap as
  gy    = vert5_anti . horiz5_sym  (gray)      banded 128x128 matmul on PE)
  msq   = gx^2+gy^2 ; NMS compares run on msq (monotone equiv of |grad|)
  direction class from gx^2,gy^2,sign(gx*gy); neighbor max selected with
  copy_predicated cascade; row+-1 shifts via DMA partition remap.
  mag   = sqrt(msq+1e-6); nm = keep*mag; out_u16 = (nm*SC)*sigmoid(10*nm-{3,1}).

Row tiling: 5 chunks of 124 output rows (last 16), each chunk stored on 128
partitions = rows 124t-2 .. 124t+125 (2-row vertical halo baked into the DMA
loads), so the single vertical conv stage needs no cross-tile fixups.
"""

import math
import os
import time
from concurrent.futures import ThreadPoolExecutor

import numpy as np

import concourse.mybir as mybir
from concourse import bacc
from concourse.tile import TileContext, add_dep_helper

FP = mybir.dt.float32
U16 = mybir.dt.uint16
U8 = mybir.dt.uint8
OP = mybir.AluOpType
AF = mybir.ActivationFunctionType

P = 128
W = 512
H = 512
NT = 5                    # row chunks per image
TR = 124                  # output rows per chunk (last chunk: 16)
GW = W + 4                # gpad chunk width (2-col zero pad each side)
MW = W + 2                # msq-type chunk width (1-col pad each side, -1.0)
B_PER_CORE = 4
N_CORES = 8
HX = H + 96               # x rows: 512 image rows + 96 rows carrying the f32
                          # PE weights bit-cast to u16 (4 imgs x 96 rows x 512
                          # = 2 dirs x 3 tiles x 128 x 128 f32 exactly)

XS = 65535.0              # input quant scale (gq = round(graysum * XS / 3))
SC = 255.0 / 4.0          # hi quant scale: 8-bit
SCD = 15.0 / 0.12         # diff quant scale: 4-bit; max of
                          # v*(sig(10(v-.1))-sig(10(v-.3))) over v>=0 is 0.11452
                          # (data-independent), so diff*SCD < 15

# rows_out[t], and the input row span of chunk t is 124t-2 .. 124t+125
ROWS_OUT = [124, 124, 124, 124, 16]


def _banded(n, taps):
    # correlation matrix: out[y] = sum_o taps[o+k] * in[y+o], zero pad
    k = len(taps) // 2
    m = np.zeros((n, n), np.float64)
    for o in range(-k, k + 1):
        for y in range(n):
            if 0 <= y + o < n:
                m[y, y + o] = taps[o + k]
    return m


def _consts():
    e = math.exp(-0.5)
    s = 1.0 + 2.0 * e
    a = e / s          # gauss edge tap
    b = 1.0 / s        # gauss center tap
    ag = _banded(H, [a, b, a])
    # exact composition of vertical gauss then vertical sobel taps, with the
    # reference's per-stage zero padding (border rows differ from the
    # translation-invariant 5-tap); 1/XS dequantizes the uint16 input
    wx_full = (_banded(H, [1.0, 2.0, 1.0]) @ ag) * (a / XS)
    wy_full = (_banded(H, [-1.0, 0.0, 1.0]) @ ag) * (a / XS)

    def tile_w(full, t):
        w = np.zeros((P, P), np.float64)
        for m_ in range(ROWS_OUT[t]):
            row_out = TR * t + m_
            for k_ in range(P):
                row_in = TR * t - 2 + k_
                if 0 <= row_in < H:
                    w[k_, m_] = full[row_out, row_in]
        return w.astype(np.float32)

    wgx = [tile_w(wx_full, t) for t in (0, 1, 4)]   # tiles 1..3 identical
    wgy = [tile_w(wy_full, t) for t in (0, 1, 4)]
    t1sq = math.tan(math.pi / 8.0) ** 2
    t2sq = math.tan(3.0 * math.pi / 8.0) ** 2
    return (
        wgx,
        wgy,
        np.float32(b / a),        # hgauss STT ratio (hs = (b/a)*g + (gl+gr))
        np.float32(t1sq),
        np.float32(t2sq),
    )


WGX_NP, WGY_NP, R_HG, T1SQ, T2SQ = _consts()


def build_bass():
    nc = bacc.Bacc("TRN2", target_bir_lowering=False, debug=False,
                   dynamic_dma_scratch_size=4096)

    x = nc.dram_tensor("x", [B_PER_CORE, 1, HX, W], U16, kind="ExternalInput")
    yp = nc.dram_tensor("yp", [B_PER_CORE, 3, H, W // 2], U8, kind="ExternalOutput")

    # persistent SBUF
    wgx_s = nc.alloc_sbuf_tensor("wgx_s", [P, 3 * P], FP)
    wgy_s = nc.alloc_sbuf_tensor("wgy_s", [P, 3 * P], FP)
    chanA = nc.alloc_sbuf_tensor("chanA", [P, NT, W], U16)
    gpad = nc.alloc_sbuf_tensor("gpad", [P, NT, GW], FP)
    hsp = nc.alloc_sbuf_tensor("hsp", [P, NT, MW], FP)
    sA = nc.alloc_sbuf_tensor("sA", [P, NT, W], FP)
    sB = nc.alloc_sbuf_tensor("sB", [P, NT, W + 1], FP)
    hgx = nc.alloc_sbuf_tensor("hgx", [P, NT, W], FP)   # later: w = gx*gy
    hgy = nc.alloc_sbuf_tensor("hgy", [P, NT, W], FP)   # later: s-mask
    q1 = nc.alloc_sbuf_tensor("q1", [P, NT, W], FP)     # later: sigmoid(hi)
    q2 = nc.alloc_sbuf_tensor("q2", [P, NT, W], FP)     # later: sigmoid(lo)
    msqp = nc.alloc_sbuf_tensor("msqp", [P, NT, MW], FP)
    ubuf = nc.alloc_sbuf_tensor("ubuf", [P, NT, MW], FP)
    dbuf = nc.alloc_sbuf_tensor("dbuf", [P, NT, MW], FP)
    mdmag = nc.alloc_sbuf_tensor("mdmag", [P, NT, W], FP)  # Md, then mag
    mx = nc.alloc_sbuf_tensor("mx", [P, NT, W], FP)  # M1 then M0 scratch
    cmask = nc.alloc_sbuf_tensor("cmask", [P, NT, W], mybir.dt.uint8)
    smask = nc.alloc_sbuf_tensor("smask", [P, NT, W], mybir.dt.uint8)
    nm0 = nc.alloc_sbuf_tensor("nm0", [P, NT, W], FP)
    nm1 = nc.alloc_sbuf_tensor("nm1", [P, NT, W], FP)
    # u16 output staging, duplicated per image parity so the store DMAs of
    # image i overlap the compute of image i+1
    h8e = [nc.alloc_sbuf_tensor(f"h8e{i}", [P, NT, W // 2], U8) for i in (0, 1)]
    h8o = [nc.alloc_sbuf_tensor(f"h8o{i}", [P, NT, W // 2], U8) for i in (0, 1)]
    p2b = [nc.alloc_sbuf_tensor(f"p2b{i}", [P, NT, W // 2], U8) for i in (0, 1)]
    dscr2 = nc.alloc_sbuf_tensor("dscr2", [P, NT, W // 2], U8)
    dscr3 = nc.alloc_sbuf_tensor("dscr3", [P, NT, W // 2], U8)
    negrow = nc.alloc_sbuf_tensor("negrow", [1, MW], FP)
    b_eps = nc.alloc_sbuf_tensor("b_eps", [P, 1], FP)
    b_hi = nc.alloc_sbuf_tensor("b_hi", [P, 1], FP)
    b_lo = nc.alloc_sbuf_tensor("b_lo", [P, 1], FP)

    nms = [nm0, nm1]

    with TileContext(nc) as tc:
        with tc.tile_pool(name="ps", bufs=3, space="PSUM") as psp:
            # ---- one-time init ----
            # weights ride along in x rows 512..607 (f32 bit-cast to u16):
            # img 2d+half holds partitions 64*half..64*half+63 of direction d
            for d, wsb in ((0, wgx_s), (1, wgy_s)):
                for half in (0, 1):
                    src_ap = (x[2 * d + half, 0, H:HX, :].flatten()
                              .bitcast(FP).rearrange("(p m) -> p m", p=64))
                    nc.sync.dma_start(wsb[64 * half:64 * half + 64, :], src_ap)
            nc.vector.memset(negrow[:, :], -1.0)
            nc.vector.memset(b_eps[:, :], 1e-6)
            nc.vector.memset(b_hi[:, :], -3.0)
            nc.vector.memset(b_lo[:, :], -1.0)
            # gpad: zero everything once (dead lanes of chunk 0/4 and the
            # 2-col pads stay zero forever; live center is rewritten per image)
            nc.vector.memset(gpad[:, :, :], 0.0)
            # msq-type pads: -1.0 sentinel (strictly below any msq >= 0)
            nc.vector.memset(msqp[:, :, 0:1], -1.0)
            nc.vector.memset(msqp[:, :, MW - 1:MW], -1.0)
            # hs pads: zero (horizontal conv zero-padding)
            nc.vector.memset(hsp[:, :, 0:1], 0.0)
            nc.vector.memset(hsp[:, :, MW - 1:MW], 0.0)
            # channel buffer: zero the never-DMA'd dead regions once so the
            # u16->f32 cast can run on full partition ranges
            nc.vector.memset(chanA[:, :, :], 0)

            def chan_load(img, dst):
                # chunk 0: rows 0..125 -> partitions 2..127
                nc.sync.dma_start(dst[2:128, 0, :], x[img, 0, 0:126, :])
                # chunks 1..3: rows 124t-2 .. 124t+125 (overlapping halos)
                for t in range(1, 4):
                    r0 = 124 * t - 2
                    nc.sync.dma_start(dst[:, t, :], x[img, 0, r0:r0 + 128, :])
                # chunk 4: rows 494..511 -> partitions 0..17
                nc.sync.dma_start(dst[0:18, 4, :], x[img, 0, 494:512, :])

            def phase_a(img, nm):
                # ---------------- front: gray load + horizontal 5-taps -----
                chan_load(img, chanA)
                # u16 -> f32 cast into the padded gray buffer
                nc.vector.tensor_single_scalar(
                    out=gpad[:, :, 2:514], in_=chanA[:, :, :], scalar=0,
                    op=OP.add)

                # horizontal gauss: hs = (b/a)*g + (g[-1]+g[+1]), x(a/3/XS)
                # folded into the PE weights
                nc.gpsimd.tensor_tensor(out=sA[:, :, :], in0=gpad[:, :, 3:515],
                                        in1=gpad[:, :, 1:513], op=OP.add)
                nc.vector.scalar_tensor_tensor(
                    out=hsp[:, :, 1:513], in0=gpad[:, :, 2:514],
                    scalar=float(R_HG), in1=sA[:, :, :],
                    op0=OP.mult, op1=OP.add)
                # horizontal sobel parts: hgx = hs[+1]-hs[-1],
                # hgy = hs[-1]+2hs[0]+hs[+1] via two [1,1] passes
                nc.vector.tensor_tensor(out=hgx[:, :, :], in0=hsp[:, :, 2:514],
                                        in1=hsp[:, :, 0:512], op=OP.subtract)
                nc.gpsimd.tensor_tensor(out=sB[:, :, 0:513],
                                        in0=hsp[:, :, 0:513],
                                        in1=hsp[:, :, 1:514], op=OP.add)
                nc.gpsimd.tensor_tensor(out=hgy[:, :, :], in0=sB[:, :, 0:512],
                                        in1=sB[:, :, 1:513], op=OP.add)

                # ---------------- vertical 5-taps on PE + evictions --------
                for t in range(NT):
                    wi = {0: 0, 4: 2}.get(t, 1)
                    gxp = psp.tile([P, W], FP, tag="gx")
                    gyp = psp.tile([P, W], FP, tag="gy")
                    nc.tensor.matmul(gxp[:, :], wgx_s[:, wi * P:wi * P + P],
                                     hgx[:, t, :], start=True, stop=True)
                    nc.tensor.matmul(gyp[:, :], wgy_s[:, wi * P:wi * P + P],
                                     hgy[:, t, :], start=True, stop=True)
                    nc.scalar.activation(q1[:, t, :], gxp[:, :], AF.Square)
                    nc.scalar.activation(q2[:, t, :], gyp[:, :], AF.Square)
                    # w = gx*gy (only its sign is used); DVE reads at most one
                    # PSUM operand, so stage gy through SBUF
                    nc.scalar.copy(sB[:, t, 0:512], gyp[:, :])
                    nc.vector.tensor_tensor(out=hgx[:, t, :], in0=gxp[:, :],
                                            in1=sB[:, t, 0:512], op=OP.mult)

                # ---------------- NMS on squared magnitude -----------------
                nc.vector.tensor_tensor(out=msqp[:, :, 1:513], in0=q1[:, :, :],
                                        in1=q2[:, :, :], op=OP.add)
                # s-mask: 1 where gx*gy >= 0 (diag direction d1)
                nc.vector.tensor_single_scalar(
                    out=smask[:, :, :], in_=hgx[:, :, :], scalar=0.0, op=OP.is_ge)

                # row shifts via DMA partition remap:
                # U[p]=msq[row+1], D[p]=msq[row-1]
                nc.sync.dma_start(ubuf[0:127, :, :], msqp[1:128, :, :])
                nc.sync.dma_start(ubuf[123:124, 0:4, :], msqp[0:1, 1:5, :])
                nc.sync.dma_start(ubuf[15:16, 4, :], negrow[0:1, :])
                nc.sync.dma_start(dbuf[1:128, :, :], msqp[0:127, :, :])
                nc.sync.dma_start(dbuf[0:1, 1:5, :], msqp[123:124, 0:4, :])
                nc.sync.dma_start(dbuf[0:1, 0, :], negrow[0:1, :])

                # neighbor maxes; Md initialized with the d3 diagonal pair
                nc.vector.tensor_tensor(out=mdmag[:, :, :], in0=ubuf[:, :, 0:512],
                                        in1=dbuf[:, :, 2:514], op=OP.max)  # M3
                nc.vector.tensor_tensor(out=mx[:, :, :], in0=ubuf[:, :, 2:514],
                                        in1=dbuf[:, :, 0:512], op=OP.max)  # M1
                nc.vector.copy_predicated(out=mdmag[:, :, :], mask=smask[:, :, :],
                                          data=mx[:, :, :])
                nc.vector.tensor_tensor(out=mx[:, :, :], in0=ubuf[:, :, 1:513],
                                        in1=dbuf[:, :, 1:513], op=OP.max)  # M2
                nc.vector.scalar_tensor_tensor(
                    out=cmask[:, :, :], in0=q1[:, :, :], scalar=float(T2SQ),
                    in1=q2[:, :, :], op0=OP.mult, op1=OP.is_lt)            # c2
                nc.vector.copy_predicated(out=mdmag[:, :, :], mask=cmask[:, :, :],
                                          data=mx[:, :, :])
                nc.vector.tensor_tensor(out=mx[:, :, :], in0=msqp[:, :, 2:514],
                                        in1=msqp[:, :, 0:512], op=OP.max)  # M0
                nc.vector.scalar_tensor_tensor(
                    out=cmask[:, :, :], in0=q1[:, :, :], scalar=float(T1SQ),
                    in1=q2[:, :, :], op0=OP.mult, op1=OP.is_gt)            # c0
                nc.vector.copy_predicated(out=mdmag[:, :, :], mask=cmask[:, :, :],
                                          data=mx[:, :, :])
                # keep = msq > Md
                nc.vector.tensor_tensor(out=cmask[:, :, :], in0=msqp[:, :, 1:513],
                                        in1=mdmag[:, :, :], op=OP.is_gt)
                # mag = sqrt(msq + 1e-6)  (overwrites Md)
                sqrt_i = nc.scalar.activation(mdmag[:, :, :], msqp[:, :, 1:513],
                                              AF.Sqrt, bias=b_eps[:, :])
                nc.vector.tensor_tensor(out=nm[:, :, :], in0=cmask[:, :, :],
                                        in1=mdmag[:, :, :], op=OP.mult)
                return sqrt_i

            def phase_b(img, nm):
                par = img % 2
                he = h8e[par]
                ho = h8o[par]
                p2 = p2b[par]
                sig1 = nc.scalar.activation(q1[:, :, :], nm[:, :, :], AF.Sigmoid,
                                            bias=b_hi[:, :], scale=10.0)
                sig2 = nc.scalar.activation(q2[:, :, :], nm[:, :, :], AF.Sigmoid,
                                            bias=b_lo[:, :], scale=10.0)
                # hi8 = (nm * SC) * sig1 (f32 math, RNE to u8), split even/odd
                # via strided DVE reads so the store DMAs stay contiguous
                nc.vector.scalar_tensor_tensor(
                    out=he[:, :, :], in0=nm[:, :, 0:W:2], scalar=float(SC),
                    in1=q1[:, :, 0:W:2], op0=OP.mult, op1=OP.mult)
                nc.vector.scalar_tensor_tensor(
                    out=ho[:, :, :], in0=nm[:, :, 1:W:2], scalar=float(SC),
                    in1=q1[:, :, 1:W:2], op0=OP.mult, op1=OP.mult)
                # d4 = (nm * SCD) * (sig2 - sig1), 4-bit; lo = hi + d4/SCD on host
                nc.gpsimd.tensor_tensor(out=sB[:, :, 0:512], in0=q2[:, :, :],
                                        in1=q1[:, :, :], op=OP.subtract)
                nc.vector.scalar_tensor_tensor(
                    out=dscr2[:, :, :], in0=nm[:, :, 0:W:2], scalar=float(SCD),
                    in1=sB[:, :, 0:W:2], op0=OP.mult, op1=OP.mult)
                nc.vector.scalar_tensor_tensor(
                    out=dscr3[:, :, :], in0=nm[:, :, 1:W:2], scalar=float(SCD),
                    in1=sB[:, :, 1:W:2], op0=OP.mult, op1=OP.mult)
                # p2 = (d4_even << 4) | d4_odd
                nc.vector.tensor_single_scalar(
                    out=dscr2[:, :, :], in_=dscr2[:, :, :], scalar=4,
                    op=OP.logical_shift_left)
                nc.vector.tensor_tensor(out=p2[:, :, :], in0=dscr2[:, :, :],
                                        in1=dscr3[:, :, :], op=OP.bitwise_or)
                # store: per plane, chunks 0..3 then chunk 4
                nc.sync.dma_start(
                    yp[img, 0, 0:496, :].rearrange("(t p) w -> p t w", p=TR),
                    he[0:124, 0:4, :])
                nc.sync.dma_start(yp[img, 0, 496:512, :], he[0:16, 4, :])
                nc.sync.dma_start(
                    yp[img, 1, 0:496, :].rearrange("(t p) w -> p t w", p=TR),
                    ho[0:124, 0:4, :])
                nc.sync.dma_start(yp[img, 1, 496:512, :], ho[0:16, 4, :])
                nc.sync.dma_start(
                    yp[img, 2, 0:496, :].rearrange("(t p) w -> p t w", p=TR),
                    p2[0:124, 0:4, :])
                nc.sync.dma_start(yp[img, 2, 496:512, :], p2[0:16, 4, :])
                return sig1, sig2

            # pairs of images share one sqrt->sigmoid table transition; deps
            # force the ACT engine to batch sqrts before sigmoids per pair so
            # walrus inserts only 4 table loads total
            prev_sig = None
            for pair in range(B_PER_CORE // 2):
                sq0 = phase_a(2 * pair, nms[0])
                if prev_sig is not None:
                    add_dep_helper(sq0.ins, prev_sig.ins, sync=False,
                                   reason="act table phase order")
                sq1 = phase_a(2 * pair + 1, nms[1])
                s0a, s0b = phase_b(2 * pair, nms[0])
                add_dep_helper(s0a.ins, sq1.ins, sync=False,
                               reason="act table phase order")
                s1a, s1b = phase_b(2 * pair + 1, nms[1])
                prev_sig = s1b

    nc.compile()
    return nc


_NC_CACHE = None


def _get_nc():
    global _NC_CACHE
    if _NC_CACHE is None:
        _NC_CACHE = build_bass()
    return _NC_CACHE


_POOL = ThreadPoolExecutor(max_workers=8)


def _weight_rows():
    # SBUF layout [partition k, (i m)] per direction, flattened f32 -> u16
    wx = np.stack(WGX_NP).transpose(1, 0, 2).reshape(P, 3 * P)
    wy = np.stack(WGY_NP).transpose(1, 0, 2).reshape(P, 3 * P)
    wb = np.concatenate([wx.reshape(-1), wy.reshape(-1)]).astype(np.float32)
    return wb.view(np.uint16).reshape(B_PER_CORE, 96, W)


# persistent input buffer: weight rows are constant, image rows rewritten
_XQ = np.empty((N_CORES * B_PER_CORE, 1, HX, W), np.uint16)
_XQ[:, 0, H:HX, :] = np.tile(_weight_rows(), (N_CORES, 1, 1)).reshape(
    N_CORES * B_PER_CORE, 96, W)


def _quantize_input(x):
    # channel-sum -> uint16 gq = round(sum * XS / 3); per-core in parallel
    def work(c):
        s = slice(c * B_PER_CORE, (c + 1) * B_PER_CORE)
        t = x[s, 0] + x[s, 1]
        np.add(t, x[s, 2], out=t)
        np.multiply(t, XS / 3.0, out=t)
        np.add(t, 0.5, out=t)
        _XQ[s, 0, 0:H, :] = t.astype(np.uint16)

    list(_POOL.map(work, range(N_CORES)))
    return _XQ


# decode LUTs: u8 code -> f32 value (single-gather decode paths)
_LUT_H = (np.arange(256, dtype=np.float32) * (1.0 / SC))
_LUT_DE = ((np.arange(256) >> 4).astype(np.float32) * (1.0 / SCD))
_LUT_DO = ((np.arange(256) & 15).astype(np.float32) * (1.0 / SCD))


def _dequantize(parts):
    # yp u8 [B,3,H,W/2] per core -> (hi, lo) f32
    # planes: hi8 evens, hi8 odds, (d4_even << 4) | d4_odd
    hi = np.empty((N_CORES * B_PER_CORE, 1, H, W), np.float32)
    lo = np.empty((N_CORES * B_PER_CORE, 1, H, W), np.float32)

    def work(c):
        s = slice(c * B_PER_CORE, (c + 1) * B_PER_CORE)
        p = parts[c]
        he = _LUT_H[p[:, 0]]
        ho = _LUT_H[p[:, 1]]
        # interleave even/odd planes in one pass each
        np.stack([he, ho], axis=-1, out=hi[s, 0].reshape(B_PER_CORE, H, W // 2, 2))
        de = _LUT_DE[p[:, 2]]
        do = _LUT_DO[p[:, 2]]
        np.add(de, he, out=de)
        np.add(do, ho, out=do)
        np.stack([de, do], axis=-1, out=lo[s, 0].reshape(B_PER_CORE, H, W // 2, 2))

    list(_POOL.map(work, range(N_CORES)))
    return hi, lo


_TIMED = bool(int(os.environ.get("KTIME", "0")))


def kernel(x: np.ndarray):
    from concourse import bass_utils

    t0 = time.perf_counter()
    x = np.asarray(x)
    assert x.shape == (32, 3, H, W), x.shape
    if x.dtype != np.float32:
        x = x.astype(np.float32)
    nc = _get_nc()
    xq = _quantize_input(x)
    in_maps = []
    for c in range(N_CORES):
        in_maps.append({
            "x": xq[c * B_PER_CORE:(c + 1) * B_PER_CORE],
        })
    t1 = time.perf_counter()
    res = bass_utils.run_bass_kernel_spmd(nc, in_maps,
                                          core_ids=list(range(N_CORES)))
    t2 = time.perf_counter()
    hi, lo = _dequantize([r["yp"] for r in res.results])
    t3 = time.perf_counter()
    if _TIMED:
        print(f"[ktime] quant {1e3*(t1-t0):.0f} ms | run {1e3*(t2-t1):.0f} ms"
              f" | dequant {1e3*(t3-t2):.0f} ms")
    return hi, lo


# revision 15
# speedup vs baseline: 1.0219x; 1.0219x over previous
"""Differentiable Canny edge detector on 8 Trainium2 NeuronCores.

Sharding: pure data parallel over batch (32 images -> 4 per core).

Wire format (the axon tunnel is the bottleneck, ~45 MB/s host->device,
~30 MB/s device->host, ~74 ms fixed cost per array):
  The kernel only ever reads the channel-sum of x (the module's first op
  is a channel mean), so the host ships the reduced sufficient statistic:
  gq = round((c0+c1+c2) * 65535/3) as uint16 (16.8 MB instead of 100.7).
  The 1/65535 dequant is folded into the PE conv weights, so gx/gy come
  out in original units and everything downstream runs on device
  unchanged. The f32 PE weights ride along bit-cast into 96 extra u16
  rows appended to each image (one DRAM input tensor total). Outputs are packed to 1.5 bytes/pixel in one u8 tensor
  [B,3,H,W/2]: plane0/plane1 = hi8 = round(hi * 255/4) at even/odd
  columns, plane2 = (d4_even << 4) | d4_odd with d4 = round((lo - hi) *
  15/0.12) (lo - hi <= 0.11452 analytically). Host decodes hi = hi8/SC,
  lo = hi + d4/SCD. Measured end-to-end L2 ~1.0e-2 (threshold 2e-2),
  dominated by hi8 rounding + rare NMS comparison flips from input
  quantization. Device f32->uint conversions are RNE (verified on
  hardware), matching the np.round model.

Per-core pipeline (per image):
  gray  = gq / 65535                          (dequant folded into PE weights)
  gx    = vert5_sym  . horiz5_anti (gray)     (horiz 5-tap on DVE, vert 5-tap as
  gy    = vert5_anti . horiz5_sym  (gray)      banded 128x128 matmul on PE)
  msq   = gx^2+gy^2 ; NMS compares run on msq (monotone equiv of |grad|)
  direction class from gx^2,gy^2,sign(gx*gy); neighbor max selected with
  copy_predicated cascade; row+-1 shifts via DMA partition remap.
  mag   = sqrt(msq+1e-6); nm = keep*mag; out_u16 = (nm*SC)*sigmoid(10*nm-{3,1}).

Row tiling: 5 chunks of 124 output rows (last 16), each chunk stored on 128
partitions = rows 124t-2 .. 124t+125 (2-row vertical halo baked into the DMA
loads), so the single vertical conv stage needs no cross-tile fixups.
"""

import math
import os
import time
from concurrent.futures import ThreadPoolExecutor

import numpy as np

import concourse.mybir as mybir
from concourse import bacc
from concourse.tile import TileContext, add_dep_helper

FP = mybir.dt.float32
U16 = mybir.dt.uint16
U8 = mybir.dt.uint8
OP = mybir.AluOpType
AF = mybir.ActivationFunctionType

P = 128
W = 512
H = 512
NT = 5                    # row chunks per image
TR = 124                  # output rows per chunk (last chunk: 16)
GW = W + 4                # gpad chunk width (2-col zero pad each side)
MW = W + 2                # msq-type chunk width (1-col pad each side, -1.0)
B_PER_CORE = 4
N_CORES = 8
HX = H + 96               # x rows: 512 image rows + 96 rows carrying the f32
                          # PE weights bit-cast to u16 (4 imgs x 96 rows x 512
                          # = 2 dirs x 3 tiles x 128 x 128 f32 exactly)

XS = 65535.0              # input quant scale (gq = round(graysum * XS / 3))
SC = 255.0 / 4.0          # hi quant scale: 8-bit
SCD = 15.0 / 0.12         # diff quant scale: 4-bit; max of
                          # v*(sig(10(v-.1))-sig(10(v-.3))) over v>=0 is 0.11452
                          # (data-independent), so diff*SCD < 15

# rows_out[t], and the input row span of chunk t is 124t-2 .. 124t+125
ROWS_OUT = [124, 124, 124, 124, 16]


def _banded(n, taps):
    # correlation matrix: out[y] = sum_o taps[o+k] * in[y+o], zero pad
    k = len(taps) // 2
    m = np.zeros((n, n), np.float64)
    for o in range(-k, k + 1):
        for y in range(n):
            if 0 <= y + o < n:
                m[y, y + o] = taps[o + k]
    return m


def _consts():
    e = math.exp(-0.5)
    s = 1.0 + 2.0 * e
    a = e / s          # gauss edge tap
    b = 1.0 / s        # gauss center tap
    ag = _banded(H, [a, b, a])
    # exact composition of vertical gauss then vertical sobel taps, with the
    # reference's per-stage zero padding (border rows differ from the
    # translation-invariant 5-tap); 1/XS dequantizes the uint16 input
    wx_full = (_banded(H, [1.0, 2.0, 1.0]) @ ag) * (a / XS)
    wy_full = (_banded(H, [-1.0, 0.0, 1.0]) @ ag) * (a / XS)

    def tile_w(full, t):
        w = np.zeros((P, P), np.float64)
        for m_ in range(ROWS_OUT[t]):
            row_out = TR * t + m_
            for k_ in range(P):
                row_in = TR * t - 2 + k_
                if 0 <= row_in < H:
                    w[k_, m_] = full[row_out, row_in]
        return w.astype(np.float32)

    wgx = [tile_w(wx_full, t) for t in (0, 1, 4)]   # tiles 1..3 identical
    wgy = [tile_w(wy_full, t) for t in (0, 1, 4)]
    t1sq = math.tan(math.pi / 8.0) ** 2
    t2sq = math.tan(3.0 * math.pi / 8.0) ** 2
    return (
        wgx,
        wgy,
        np.float32(b / a),        # hgauss STT ratio (hs = (b/a)*g + (gl+gr))
        np.float32(t1sq),
        np.float32(t2sq),
    )


WGX_NP, WGY_NP, R_HG, T1SQ, T2SQ = _consts()


def build_bass():
    nc = bacc.Bacc("TRN2", target_bir_lowering=False, debug=False,
                   dynamic_dma_scratch_size=4096)

    x = nc.dram_tensor("x", [B_PER_CORE, 1, HX, W], U16, kind="ExternalInput")
    yp = nc.dram_tensor("yp", [B_PER_CORE, 3, H, W // 2], U8, kind="ExternalOutput")

    # persistent SBUF
    wgx_s = nc.alloc_sbuf_tensor("wgx_s", [P, 3 * P], FP)
    wgy_s = nc.alloc_sbuf_tensor("wgy_s", [P, 3 * P], FP)
    chanA = nc.alloc_sbuf_tensor("chanA", [P, NT, W], U16)
    gpad = nc.alloc_sbuf_tensor("gpad", [P, NT, GW], FP)
    hsp = nc.alloc_sbuf_tensor("hsp", [P, NT, MW], FP)
    sA = nc.alloc_sbuf_tensor("sA", [P, NT, W], FP)
    sB = nc.alloc_sbuf_tensor("sB", [P, NT, W + 1], FP)
    hgx = nc.alloc_sbuf_tensor("hgx", [P, NT, W], FP)   # later: w = gx*gy
    hgy = nc.alloc_sbuf_tensor("hgy", [P, NT, W], FP)   # later: s-mask
    q1 = nc.alloc_sbuf_tensor("q1", [P, NT, W], FP)     # later: sigmoid(hi)
    q2 = nc.alloc_sbuf_tensor("q2", [P, NT, W], FP)     # later: sigmoid(lo)
    msqp = nc.alloc_sbuf_tensor("msqp", [P, NT, MW], FP)
    ubuf = nc.alloc_sbuf_tensor("ubuf", [P, NT, MW], FP)
    dbuf = nc.alloc_sbuf_tensor("dbuf", [P, NT, MW], FP)
    mdmag = nc.alloc_sbuf_tensor("mdmag", [P, NT, W], FP)  # Md, then mag
    mx = nc.alloc_sbuf_tensor("mx", [P, NT, W], FP)  # M1 then M0 scratch
    cmask = nc.alloc_sbuf_tensor("cmask", [P, NT, W], mybir.dt.uint8)
    smask = nc.alloc_sbuf_tensor("smask", [P, NT, W], mybir.dt.uint8)
    nm0 = nc.alloc_sbuf_tensor("nm0", [P, NT, W], FP)
    nm1 = nc.alloc_sbuf_tensor("nm1", [P, NT, W], FP)
    # u16 output staging, duplicated per image parity so the store DMAs of
    # image i overlap the compute of image i+1
    h8e = [nc.alloc_sbuf_tensor(f"h8e{i}", [P, NT, W // 2], U8) for i in (0, 1)]
    h8o = [nc.alloc_sbuf_tensor(f"h8o{i}", [P, NT, W // 2], U8) for i in (0, 1)]
    p2b = [nc.alloc_sbuf_tensor(f"p2b{i}", [P, NT, W // 2], U8) for i in (0, 1)]
    dscr2 = nc.alloc_sbuf_tensor("dscr2", [P, NT, W // 2], U8)
    dscr3 = nc.alloc_sbuf_tensor("dscr3", [P, NT, W // 2], U8)
    negrow = nc.alloc_sbuf_tensor("negrow", [1, MW], FP)
    b_eps = nc.alloc_sbuf_tensor("b_eps", [P, 1], FP)
    b_hi = nc.alloc_sbuf_tensor("b_hi", [P, 1], FP)
    b_lo = nc.alloc_sbuf_tensor("b_lo", [P, 1], FP)

    nms = [nm0, nm1]

    with TileContext(nc) as tc:
        with tc.tile_pool(name="ps", bufs=3, space="PSUM") as psp:
            # ---- one-time init ----
            # weights ride along in x rows 512..607 (f32 bit-cast to u16):
            # img 2d+half holds partitions 64*half..64*half+63 of direction d
            for d, wsb in ((0, wgx_s), (1, wgy_s)):
                for half in (0, 1):
                    src_ap = (x[2 * d + half, 0, H:HX, :].flatten()
                              .bitcast(FP).rearrange("(p m) -> p m", p=64))
                    nc.sync.dma_start(wsb[64 * half:64 * half + 64, :], src_ap)
            nc.vector.memset(negrow[:, :], -1.0)
            nc.vector.memset(b_eps[:, :], 1e-6)
            nc.vector.memset(b_hi[:, :], -3.0)
            nc.vector.memset(b_lo[:, :], -1.0)
            # gpad: zero everything once (dead lanes of chunk 0/4 and the
            # 2-col pads stay zero forever; live center is rewritten per image)
            nc.vector.memset(gpad[:, :, :], 0.0)
            # msq-type pads: -1.0 sentinel (strictly below any msq >= 0)
            nc.vector.memset(msqp[:, :, 0:1], -1.0)
            nc.vector.memset(msqp[:, :, MW - 1:MW], -1.0)
            # hs pads: zero (horizontal conv zero-padding)
            nc.vector.memset(hsp[:, :, 0:1], 0.0)
            nc.vector.memset(hsp[:, :, MW - 1:MW], 0.0)
            # channel buffer: zero the never-DMA'd dead regions once so the
            # u16->f32 cast can run on full partition ranges
            nc.vector.memset(chanA[:, :, :], 0)

            def chan_load(img, dst):
                # chunk 0: rows 0..125 -> partitions 2..127
                nc.sync.dma_start(dst[2:128, 0, :], x[img, 0, 0:126, :])
                # chunks 1..3: rows 124t-2 .. 124t+125 (overlapping halos)
                for t in range(1, 4):
                    r0 = 124 * t - 2
                    nc.sync.dma_start(dst[:, t, :], x[img, 0, r0:r0 + 128, :])
                # chunk 4: rows 494..511 -> partitions 0..17
                nc.sync.dma_start(dst[0:18, 4, :], x[img, 0, 494:512, :])

            def phase_a(img, nm):
                # ---------------- front: gray load + horizontal 5-taps -----
                chan_load(img, chanA)
                # u16 -> f32 cast into the padded gray buffer
                nc.vector.tensor_single_scalar(
                    out=gpad[:, :, 2:514], in_=chanA[:, :, :], scalar=0,
                    op=OP.add)

                # horizontal gauss: hs = (b/a)*g + (g[-1]+g[+1]), x(a/3/XS)
                # folded into the PE weights
                nc.gpsimd.tensor_tensor(out=sA[:, :, :], in0=gpad[:, :, 3:515],
                                        in1=gpad[:, :, 1:513], op=OP.add)
                nc.vector.scalar_tensor_tensor(
                    out=hsp[:, :, 1:513], in0=gpad[:, :, 2:514],
                    scalar=float(R_HG), in1=sA[:, :, :],
                    op0=OP.mult, op1=OP.add)
                # horizontal sobel parts: hgx = hs[+1]-hs[-1],
                # hgy = hs[-1]+2hs[0]+hs[+1] via two [1,1] passes
                nc.vector.tensor_tensor(out=hgx[:, :, :], in0=hsp[:, :, 2:514],
                                        in1=hsp[:, :, 0:512], op=OP.subtract)
                nc.gpsimd.tensor_tensor(out=sB[:, :, 0:513],
                                        in0=hsp[:, :, 0:513],
                                        in1=hsp[:, :, 1:514], op=OP.add)
                nc.gpsimd.tensor_tensor(out=hgy[:, :, :], in0=sB[:, :, 0:512],
                                        in1=sB[:, :, 1:513], op=OP.add)

                # ---------------- vertical 5-taps on PE + evictions --------
                for t in range(NT):
                    wi = {0: 0, 4: 2}.get(t, 1)
                    gxp = psp.tile([P, W], FP, tag="gx")
                    gyp = psp.tile([P, W], FP, tag="gy")
                    nc.tensor.matmul(gxp[:, :], wgx_s[:, wi * P:wi * P + P],
                                     hgx[:, t, :], start=True, stop=True)
                    nc.tensor.matmul(gyp[:, :], wgy_s[:, wi * P:wi * P + P],
                                     hgy[:, t, :], start=True, stop=True)
                    nc.scalar.activation(q1[:, t, :], gxp[:, :], AF.Square)
                    nc.scalar.activation(q2[:, t, :], gyp[:, :], AF.Square)
                    # w = gx*gy (only its sign is used); DVE reads at most one
                    # PSUM operand, so stage gy through SBUF
                    nc.scalar.copy(sB[:, t, 0:512], gyp[:, :])
                    nc.vector.tensor_tensor(out=hgx[:, t, :], in0=gxp[:, :],
                                            in1=sB[:, t, 0:512], op=OP.mult)

                # ---------------- NMS on squared magnitude -----------------
                nc.vector.tensor_tensor(out=msqp[:, :, 1:513], in0=q1[:, :, :],
                                        in1=q2[:, :, :], op=OP.add)
                # s-mask: 1 where gx*gy >= 0 (diag direction d1)
                nc.vector.tensor_single_scalar(
                    out=smask[:, :, :], in_=hgx[:, :, :], scalar=0.0, op=OP.is_ge)

                # row shifts via DMA partition remap:
                # U[p]=msq[row+1], D[p]=msq[row-1]
                nc.sync.dma_start(ubuf[0:127, :, :], msqp[1:128, :, :])
                nc.sync.dma_start(ubuf[123:124, 0:4, :], msqp[0:1, 1:5, :])
                nc.sync.dma_start(ubuf[15:16, 4, :], negrow[0:1, :])
                nc.sync.dma_start(dbuf[1:128, :, :], msqp[0:127, :, :])
                nc.sync.dma_start(dbuf[0:1, 1:5, :], msqp[123:124, 0:4, :])
                nc.sync.dma_start(dbuf[0:1, 0, :], negrow[0:1, :])

                # neighbor maxes; Md initialized with the d3 diagonal pair
                nc.vector.tensor_tensor(out=mdmag[:, :, :], in0=ubuf[:, :, 0:512],
                                        in1=dbuf[:, :, 2:514], op=OP.max)  # M3
                nc.vector.tensor_tensor(out=mx[:, :, :], in0=ubuf[:, :, 2:514],
                                        in1=dbuf[:, :, 0:512], op=OP.max)  # M1
                nc.vector.copy_predicated(out=mdmag[:, :, :], mask=smask[:, :, :],
                                          data=mx[:, :, :])
                nc.vector.tensor_tensor(out=mx[:, :, :], in0=ubuf[:, :, 1:513],
                                        in1=dbuf[:, :, 1:513], op=OP.max)  # M2
                nc.vector.scalar_tensor_tensor(
                    out=cmask[:, :, :], in0=q1[:, :, :], scalar=float(T2SQ),
                    in1=q2[:, :, :], op0=OP.mult, op1=OP.is_lt)            # c2
                nc.vector.copy_predicated(out=mdmag[:, :, :], mask=cmask[:, :, :],
                                          data=mx[:, :, :])
                nc.vector.tensor_tensor(out=mx[:, :, :], in0=msqp[:, :, 2:514],
                                        in1=msqp[:, :, 0:512], op=OP.max)  # M0
                nc.vector.scalar_tensor_tensor(
                    out=cmask[:, :, :], in0=q1[:, :, :], scalar=float(T1SQ),
                    in1=q2[:, :, :], op0=OP.mult, op1=OP.is_gt)            # c0
                nc.vector.copy_predicated(out=mdmag[:, :, :], mask=cmask[:, :, :],
                                          data=mx[:, :, :])
                # keep = msq > Md
                nc.vector.tensor_tensor(out=cmask[:, :, :], in0=msqp[:, :, 1:513],
                                        in1=mdmag[:, :, :], op=OP.is_gt)
                # mag = sqrt(msq + 1e-6)  (overwrites Md)
                sqrt_i = nc.scalar.activation(mdmag[:, :, :], msqp[:, :, 1:513],
                                              AF.Sqrt, bias=b_eps[:, :])
                nc.vector.tensor_tensor(out=nm[:, :, :], in0=cmask[:, :, :],
                                        in1=mdmag[:, :, :], op=OP.mult)
                return sqrt_i

            def phase_b(img, nm):
                par = img % 2
                he = h8e[par]
                ho = h8o[par]
                p2 = p2b[par]
                sig1 = nc.scalar.activation(q1[:, :, :], nm[:, :, :], AF.Sigmoid,
                                            bias=b_hi[:, :], scale=10.0)
                sig2 = nc.scalar.activation(q2[:, :, :], nm[:, :, :], AF.Sigmoid,
                                            bias=b_lo[:, :], scale=10.0)
                # hi8 = (nm * SC) * sig1 (f32 math, RNE to u8), split even/odd
                # via strided DVE reads so the store DMAs stay contiguous
                nc.vector.scalar_tensor_tensor(
                    out=he[:, :, :], in0=nm[:, :, 0:W:2], scalar=float(SC),
                    in1=q1[:, :, 0:W:2], op0=OP.mult, op1=OP.mult)
                nc.vector.scalar_tensor_tensor(
                    out=ho[:, :, :], in0=nm[:, :, 1:W:2], scalar=float(SC),
                    in1=q1[:, :, 1:W:2], op0=OP.mult, op1=OP.mult)
                # d4 = (nm * SCD) * (sig2 - sig1), 4-bit; lo = hi + d4/SCD on host
                nc.gpsimd.tensor_tensor(out=sB[:, :, 0:512], in0=q2[:, :, :],
                                        in1=q1[:, :, :], op=OP.subtract)
                nc.vector.scalar_tensor_tensor(
                    out=dscr2[:, :, :], in0=nm[:, :, 0:W:2], scalar=float(SCD),
                    in1=sB[:, :, 0:W:2], op0=OP.mult, op1=OP.mult)
                nc.vector.scalar_tensor_tensor(
                    out=dscr3[:, :, :], in0=nm[:, :, 1:W:2], scalar=float(SCD),
                    in1=sB[:, :, 1:W:2], op0=OP.mult, op1=OP.mult)
                # p2 = (d4_even << 4) | d4_odd
                nc.vector.tensor_single_scalar(
                    out=dscr2[:, :, :], in_=dscr2[:, :, :], scalar=4,
                    op=OP.logical_shift_left)
                nc.vector.tensor_tensor(out=p2[:, :, :], in0=dscr2[:, :, :],
                                        in1=dscr3[:, :, :], op=OP.bitwise_or)
                # store: per plane, chunks 0..3 then chunk 4
                nc.sync.dma_start(
                    yp[img, 0, 0:496, :].rearrange("(t p) w -> p t w", p=TR),
                    he[0:124, 0:4, :])
                nc.sync.dma_start(yp[img, 0, 496:512, :], he[0:16, 4, :])
                nc.sync.dma_start(
                    yp[img, 1, 0:496, :].rearrange("(t p) w -> p t w", p=TR),
                    ho[0:124, 0:4, :])
                nc.sync.dma_start(yp[img, 1, 496:512, :], ho[0:16, 4, :])
                nc.sync.dma_start(
                    yp[img, 2, 0:496, :].rearrange("(t p) w -> p t w", p=TR),
                    p2[0:124, 0:4, :])
                nc.sync.dma_start(yp[img, 2, 496:512, :], p2[0:16, 4, :])
                return sig1, sig2

            # pairs of images share one sqrt->sigmoid table transition; deps
            # force the ACT engine to batch sqrts before sigmoids per pair so
            # walrus inserts only 4 table loads total
            prev_sig = None
            for pair in range(B_PER_CORE // 2):
                sq0 = phase_a(2 * pair, nms[0])
                if prev_sig is not None:
                    add_dep_helper(sq0.ins, prev_sig.ins, sync=False,
                                   reason="act table phase order")
                sq1 = phase_a(2 * pair + 1, nms[1])
                s0a, s0b = phase_b(2 * pair, nms[0])
                add_dep_helper(s0a.ins, sq1.ins, sync=False,
                               reason="act table phase order")
                s1a, s1b = phase_b(2 * pair + 1, nms[1])
                prev_sig = s1b

    nc.compile()
    return nc


_NC_CACHE = None


def _get_nc():
    global _NC_CACHE
    if _NC_CACHE is None:
        _NC_CACHE = build_bass()
    return _NC_CACHE


_POOL = ThreadPoolExecutor(max_workers=8)


def _weight_rows():
    # SBUF layout [partition k, (i m)] per direction, flattened f32 -> u16
    wx = np.stack(WGX_NP).transpose(1, 0, 2).reshape(P, 3 * P)
    wy = np.stack(WGY_NP).transpose(1, 0, 2).reshape(P, 3 * P)
    wb = np.concatenate([wx.reshape(-1), wy.reshape(-1)]).astype(np.float32)
    return wb.view(np.uint16).reshape(B_PER_CORE, 96, W)


# persistent input buffer: weight rows are constant, image rows rewritten
_XQ = np.empty((N_CORES * B_PER_CORE, 1, HX, W), np.uint16)
_XQ[:, 0, H:HX, :] = np.tile(_weight_rows(), (N_CORES, 1, 1)).reshape(
    N_CORES * B_PER_CORE, 96, W)


def _quantize_input(x):
    # channel-sum -> uint16 gq = round(sum * XS / 3); per-core in parallel
    def work(c):
        s = slice(c * B_PER_CORE, (c + 1) * B_PER_CORE)
        t = x[s, 0] + x[s, 1]
        np.add(t, x[s, 2], out=t)
        np.multiply(t, XS / 3.0, out=t)
        np.add(t, 0.5, out=t)
        _XQ[s, 0, 0:H, :] = t.astype(np.uint16)

    list(_POOL.map(work, range(N_CORES)))
    return _XQ


def _dequantize(parts):
    # yp u8 [B,3,H,W/2] per core -> (hi, lo) f32
    # planes: hi8 evens, hi8 odds, (d4_even << 4) | d4_odd
    hi = np.empty((N_CORES * B_PER_CORE, 1, H, W), np.float32)
    lo = np.empty((N_CORES * B_PER_CORE, 1, H, W), np.float32)

    def work(c):
        s = slice(c * B_PER_CORE, (c + 1) * B_PER_CORE)
        p = parts[c]
        he = p[:, 0].astype(np.float32)
        ho = p[:, 1].astype(np.float32)
        np.multiply(he, 1.0 / SC, out=he)
        np.multiply(ho, 1.0 / SC, out=ho)
        # interleave even/odd planes in one pass each
        np.stack([he, ho], axis=-1, out=hi[s, 0].reshape(B_PER_CORE, H, W // 2, 2))
        de = (p[:, 2] >> 4).astype(np.float32)
        do = (p[:, 2] & 15).astype(np.float32)
        np.multiply(de, 1.0 / SCD, out=de)
        np.multiply(do, 1.0 / SCD, out=do)
        np.add(de, he, out=de)
        np.add(do, ho, out=do)
        np.stack([de, do], axis=-1, out=lo[s, 0].reshape(B_PER_CORE, H, W // 2, 2))

    list(_POOL.map(work, range(N_CORES)))
    return hi, lo


_TIMED = bool(int(os.environ.get("KTIME", "0")))


def kernel(x: np.ndarray):
    from concourse import bass_utils

    t0 = time.perf_counter()
    x = np.asarray(x)
    assert x.shape == (32, 3, H, W), x.shape
    if x.dtype != np.float32:
        x = x.astype(np.float32)
    nc = _get_nc()
    xq = _quantize_input(x)
    in_maps = []
    for c in range(N_CORES):
        in_maps.append({
            "x": xq[c * B_PER_CORE:(c + 1) * B_PER_CORE],
        })
    t1 = time.perf_counter()
    res = bass_utils.run_bass_kernel_spmd(nc, in_maps,
                                          core_ids=list(range(N_CORES)))
    t2 = time.perf_counter()
    hi, lo = _dequantize([r["yp"] for r in res.results])
    t3 = time.perf_counter()
    if _TIMED:
        print(f"[ktime] quant {1e3*(t1-t0):.0f} ms | run {1e3*(t2-t1):.0f} ms"
              f" | dequant {1e3*(t3-t2):.0f} ms")
    return hi, lo


# revision 16
# speedup vs baseline: 1.2225x; 1.1963x over previous
"""Differentiable Canny edge detector on 8 Trainium2 NeuronCores.

Sharding: pure data parallel over batch (32 images -> 4 per core).

Wire format (the axon tunnel is the bottleneck, ~45 MB/s host->device,
~30 MB/s device->host, ~74 ms fixed cost per array):
  The kernel only ever reads the channel-sum of x (the module's first op
  is a channel mean), so the host ships the reduced sufficient statistic:
  gq = round((c0+c1+c2) * 65535/3) as uint16 (16.8 MB instead of 100.7).
  The 1/65535 dequant is folded into the PE conv weights, so gx/gy come
  out in original units and everything downstream runs on device
  unchanged. The f32 PE weights ride along bit-cast into 96 extra u16
  rows appended to each image (one DRAM input tensor total). Outputs are packed to 1.5 bytes/pixel in one u8 tensor
  [B,3,H,W/2]: plane0/plane1 = hi8 = round(hi * 255/4) at even/odd
  columns, plane2 = (d4_even << 4) | d4_odd with d4 = round((lo - hi) *
  15/0.12) (lo - hi <= 0.11452 analytically). Host decodes hi = hi8/SC,
  lo = hi + d4/SCD. Measured end-to-end L2 ~1.0e-2 (threshold 2e-2),
  dominated by hi8 rounding + rare NMS comparison flips from input
  quantization. Device f32->uint conversions are RNE (verified on
  hardware), matching the np.round model.

Per-core pipeline (per image):
  gray  = gq / 65535                          (dequant folded into PE weights)
  gx    = vert5_sym  . horiz5_anti (gray)     (horiz 5-tap on DVE, vert 5-tap as
  gy    = vert5_anti . horiz5_sym  (gray)      banded 128x128 matmul on PE)
  msq   = gx^2+gy^2 ; NMS compares run on msq (monotone equiv of |grad|)
  direction class from gx^2,gy^2,sign(gx*gy); neighbor max selected with
  copy_predicated cascade; row+-1 shifts via DMA partition remap.
  mag   = sqrt(msq+1e-6); nm = keep*mag; out_u16 = (nm*SC)*sigmoid(10*nm-{3,1}).

Row tiling: 5 chunks of 124 output rows (last 16), each chunk stored on 128
partitions = rows 124t-2 .. 124t+125 (2-row vertical halo baked into the DMA
loads), so the single vertical conv stage needs no cross-tile fixups.
"""

import math
import os
import time
from concurrent.futures import ThreadPoolExecutor

import numpy as np

import concourse.mybir as mybir
from concourse import bacc
from concourse.tile import TileContext, add_dep_helper

FP = mybir.dt.float32
U16 = mybir.dt.uint16
U8 = mybir.dt.uint8
OP = mybir.AluOpType
AF = mybir.ActivationFunctionType

P = 128
W = 512
H = 512
NT = 5                    # row chunks per image
TR = 124                  # output rows per chunk (last chunk: 16)
GW = W + 4                # gpad chunk width (2-col zero pad each side)
MW = W + 2                # msq-type chunk width (1-col pad each side, -1.0)
B_PER_CORE = 4
N_CORES = 8
HX = H + 96               # x rows: 512 image rows + 96 rows carrying the f32
                          # PE weights bit-cast to u16 (4 imgs x 96 rows x 512
                          # = 2 dirs x 3 tiles x 128 x 128 f32 exactly)

XS = 65535.0              # input quant scale (gq = round(graysum * XS / 3))
SC = 255.0 / 4.0          # hi quant scale: 8-bit
SCD = 15.0 / 0.12         # diff quant scale: 4-bit; max of
                          # v*(sig(10(v-.1))-sig(10(v-.3))) over v>=0 is 0.11452
                          # (data-independent), so diff*SCD < 15

# rows_out[t], and the input row span of chunk t is 124t-2 .. 124t+125
ROWS_OUT = [124, 124, 124, 124, 16]


def _banded(n, taps):
    # correlation matrix: out[y] = sum_o taps[o+k] * in[y+o], zero pad
    k = len(taps) // 2
    m = np.zeros((n, n), np.float64)
    for o in range(-k, k + 1):
        for y in range(n):
            if 0 <= y + o < n:
                m[y, y + o] = taps[o + k]
    return m


def _consts():
    e = math.exp(-0.5)
    s = 1.0 + 2.0 * e
    a = e / s          # gauss edge tap
    b = 1.0 / s        # gauss center tap
    ag = _banded(H, [a, b, a])
    # exact composition of vertical gauss then vertical sobel taps, with the
    # reference's per-stage zero padding (border rows differ from the
    # translation-invariant 5-tap); 1/XS dequantizes the uint16 input
    wx_full = (_banded(H, [1.0, 2.0, 1.0]) @ ag) * (a / XS)
    wy_full = (_banded(H, [-1.0, 0.0, 1.0]) @ ag) * (a / XS)

    def tile_w(full, t):
        w = np.zeros((P, P), np.float64)
        for m_ in range(ROWS_OUT[t]):
            row_out = TR * t + m_
            for k_ in range(P):
                row_in = TR * t - 2 + k_
                if 0 <= row_in < H:
                    w[k_, m_] = full[row_out, row_in]
        return w.astype(np.float32)

    wgx = [tile_w(wx_full, t) for t in (0, 1, 4)]   # tiles 1..3 identical
    wgy = [tile_w(wy_full, t) for t in (0, 1, 4)]
    t1sq = math.tan(math.pi / 8.0) ** 2
    t2sq = math.tan(3.0 * math.pi / 8.0) ** 2
    return (
        wgx,
        wgy,
        np.float32(b / a),        # hgauss STT ratio (hs = (b/a)*g + (gl+gr))
        np.float32(t1sq),
        np.float32(t2sq),
    )


WGX_NP, WGY_NP, R_HG, T1SQ, T2SQ = _consts()


def build_bass():
    nc = bacc.Bacc("TRN2", target_bir_lowering=False, debug=False,
                   dynamic_dma_scratch_size=4096)

    x = nc.dram_tensor("x", [B_PER_CORE, 1, HX, W], U16, kind="ExternalInput")
    yp = nc.dram_tensor("yp", [B_PER_CORE, 3, H, W // 2], U8, kind="ExternalOutput")

    # persistent SBUF
    wgx_s = nc.alloc_sbuf_tensor("wgx_s", [P, 3 * P], FP)
    wgy_s = nc.alloc_sbuf_tensor("wgy_s", [P, 3 * P], FP)
    chanA = nc.alloc_sbuf_tensor("chanA", [P, NT, W], U16)
    gpad = nc.alloc_sbuf_tensor("gpad", [P, NT, GW], FP)
    hsp = nc.alloc_sbuf_tensor("hsp", [P, NT, MW], FP)
    sA = nc.alloc_sbuf_tensor("sA", [P, NT, W], FP)
    sB = nc.alloc_sbuf_tensor("sB", [P, NT, W + 1], FP)
    hgx = nc.alloc_sbuf_tensor("hgx", [P, NT, W], FP)   # later: w = gx*gy
    hgy = nc.alloc_sbuf_tensor("hgy", [P, NT, W], FP)   # later: s-mask
    q1 = nc.alloc_sbuf_tensor("q1", [P, NT, W], FP)     # later: sigmoid(hi)
    q2 = nc.alloc_sbuf_tensor("q2", [P, NT, W], FP)     # later: sigmoid(lo)
    msqp = nc.alloc_sbuf_tensor("msqp", [P, NT, MW], FP)
    ubuf = nc.alloc_sbuf_tensor("ubuf", [P, NT, MW], FP)
    dbuf = nc.alloc_sbuf_tensor("dbuf", [P, NT, MW], FP)
    mdmag = nc.alloc_sbuf_tensor("mdmag", [P, NT, W], FP)  # Md, then mag
    mx = nc.alloc_sbuf_tensor("mx", [P, NT, W], FP)  # M1 then M0 scratch
    cmask = nc.alloc_sbuf_tensor("cmask", [P, NT, W], mybir.dt.uint8)
    smask = nc.alloc_sbuf_tensor("smask", [P, NT, W], mybir.dt.uint8)
    nm0 = nc.alloc_sbuf_tensor("nm0", [P, NT, W], FP)
    nm1 = nc.alloc_sbuf_tensor("nm1", [P, NT, W], FP)
    # u16 output staging, duplicated per image parity so the store DMAs of
    # image i overlap the compute of image i+1
    h8e = [nc.alloc_sbuf_tensor(f"h8e{i}", [P, NT, W // 2], U8) for i in (0, 1)]
    h8o = [nc.alloc_sbuf_tensor(f"h8o{i}", [P, NT, W // 2], U8) for i in (0, 1)]
    p2b = [nc.alloc_sbuf_tensor(f"p2b{i}", [P, NT, W // 2], U8) for i in (0, 1)]
    dscr2 = nc.alloc_sbuf_tensor("dscr2", [P, NT, W // 2], U8)
    dscr3 = nc.alloc_sbuf_tensor("dscr3", [P, NT, W // 2], U8)
    negrow = nc.alloc_sbuf_tensor("negrow", [1, MW], FP)
    b_eps = nc.alloc_sbuf_tensor("b_eps", [P, 1], FP)
    b_hi = nc.alloc_sbuf_tensor("b_hi", [P, 1], FP)
    b_lo = nc.alloc_sbuf_tensor("b_lo", [P, 1], FP)

    nms = [nm0, nm1]

    with TileContext(nc) as tc:
        with tc.tile_pool(name="ps", bufs=3, space="PSUM") as psp:
            # ---- one-time init ----
            # weights ride along in x rows 512..607 (f32 bit-cast to u16):
            # img 2d+half holds partitions 64*half..64*half+63 of direction d
            for d, wsb in ((0, wgx_s), (1, wgy_s)):
                for half in (0, 1):
                    src_ap = (x[2 * d + half, 0, H:HX, :].flatten()
                              .bitcast(FP).rearrange("(p m) -> p m", p=64))
                    nc.sync.dma_start(wsb[64 * half:64 * half + 64, :], src_ap)
            nc.vector.memset(negrow[:, :], -1.0)
            nc.vector.memset(b_eps[:, :], 1e-6)
            nc.vector.memset(b_hi[:, :], -3.0)
            nc.vector.memset(b_lo[:, :], -1.0)
            # gpad: zero everything once (dead lanes of chunk 0/4 and the
            # 2-col pads stay zero forever; live center is rewritten per image)
            nc.vector.memset(gpad[:, :, :], 0.0)
            # msq-type pads: -1.0 sentinel (strictly below any msq >= 0)
            nc.vector.memset(msqp[:, :, 0:1], -1.0)
            nc.vector.memset(msqp[:, :, MW - 1:MW], -1.0)
            # hs pads: zero (horizontal conv zero-padding)
            nc.vector.memset(hsp[:, :, 0:1], 0.0)
            nc.vector.memset(hsp[:, :, MW - 1:MW], 0.0)
            # channel buffer: zero the never-DMA'd dead regions once so the
            # u16->f32 cast can run on full partition ranges
            nc.vector.memset(chanA[:, :, :], 0)

            def chan_load(img, dst):
                # chunk 0: rows 0..125 -> partitions 2..127
                nc.sync.dma_start(dst[2:128, 0, :], x[img, 0, 0:126, :])
                # chunks 1..3: rows 124t-2 .. 124t+125 (overlapping halos)
                for t in range(1, 4):
                    r0 = 124 * t - 2
                    nc.sync.dma_start(dst[:, t, :], x[img, 0, r0:r0 + 128, :])
                # chunk 4: rows 494..511 -> partitions 0..17
                nc.sync.dma_start(dst[0:18, 4, :], x[img, 0, 494:512, :])

            def phase_a(img, nm):
                # ---------------- front: gray load + horizontal 5-taps -----
                chan_load(img, chanA)
                # u16 -> f32 cast into the padded gray buffer
                nc.vector.tensor_single_scalar(
                    out=gpad[:, :, 2:514], in_=chanA[:, :, :], scalar=0,
                    op=OP.add)

                # horizontal gauss: hs = (b/a)*g + (g[-1]+g[+1]), x(a/3/XS)
                # folded into the PE weights
                nc.gpsimd.tensor_tensor(out=sA[:, :, :], in0=gpad[:, :, 3:515],
                                        in1=gpad[:, :, 1:513], op=OP.add)
                nc.vector.scalar_tensor_tensor(
                    out=hsp[:, :, 1:513], in0=gpad[:, :, 2:514],
                    scalar=float(R_HG), in1=sA[:, :, :],
                    op0=OP.mult, op1=OP.add)
                # horizontal sobel parts: hgx = hs[+1]-hs[-1],
                # hgy = hs[-1]+2hs[0]+hs[+1] via two [1,1] passes
                nc.vector.tensor_tensor(out=hgx[:, :, :], in0=hsp[:, :, 2:514],
                                        in1=hsp[:, :, 0:512], op=OP.subtract)
                nc.gpsimd.tensor_tensor(out=sB[:, :, 0:513],
                                        in0=hsp[:, :, 0:513],
                                        in1=hsp[:, :, 1:514], op=OP.add)
                nc.gpsimd.tensor_tensor(out=hgy[:, :, :], in0=sB[:, :, 0:512],
                                        in1=sB[:, :, 1:513], op=OP.add)

                # ---------------- vertical 5-taps on PE + evictions --------
                for t in range(NT):
                    wi = {0: 0, 4: 2}.get(t, 1)
                    gxp = psp.tile([P, W], FP, tag="gx")
                    gyp = psp.tile([P, W], FP, tag="gy")
                    nc.tensor.matmul(gxp[:, :], wgx_s[:, wi * P:wi * P + P],
                                     hgx[:, t, :], start=True, stop=True)
                    nc.tensor.matmul(gyp[:, :], wgy_s[:, wi * P:wi * P + P],
                                     hgy[:, t, :], start=True, stop=True)
                    nc.scalar.activation(q1[:, t, :], gxp[:, :], AF.Square)
                    nc.scalar.activation(q2[:, t, :], gyp[:, :], AF.Square)
                    # w = gx*gy (only its sign is used); DVE reads at most one
                    # PSUM operand, so stage gy through SBUF
                    nc.scalar.copy(sB[:, t, 0:512], gyp[:, :])
                    nc.vector.tensor_tensor(out=hgx[:, t, :], in0=gxp[:, :],
                                            in1=sB[:, t, 0:512], op=OP.mult)

                # ---------------- NMS on squared magnitude -----------------
                nc.vector.tensor_tensor(out=msqp[:, :, 1:513], in0=q1[:, :, :],
                                        in1=q2[:, :, :], op=OP.add)
                # s-mask: 1 where gx*gy >= 0 (diag direction d1)
                nc.vector.tensor_single_scalar(
                    out=smask[:, :, :], in_=hgx[:, :, :], scalar=0.0, op=OP.is_ge)

                # row shifts via DMA partition remap:
                # U[p]=msq[row+1], D[p]=msq[row-1]
                nc.sync.dma_start(ubuf[0:127, :, :], msqp[1:128, :, :])
                nc.sync.dma_start(ubuf[123:124, 0:4, :], msqp[0:1, 1:5, :])
                nc.sync.dma_start(ubuf[15:16, 4, :], negrow[0:1, :])
                nc.sync.dma_start(dbuf[1:128, :, :], msqp[0:127, :, :])
                nc.sync.dma_start(dbuf[0:1, 1:5, :], msqp[123:124, 0:4, :])
                nc.sync.dma_start(dbuf[0:1, 0, :], negrow[0:1, :])

                # neighbor maxes; Md initialized with the d3 diagonal pair
                nc.vector.tensor_tensor(out=mdmag[:, :, :], in0=ubuf[:, :, 0:512],
                                        in1=dbuf[:, :, 2:514], op=OP.max)  # M3
                nc.vector.tensor_tensor(out=mx[:, :, :], in0=ubuf[:, :, 2:514],
                                        in1=dbuf[:, :, 0:512], op=OP.max)  # M1
                nc.vector.copy_predicated(out=mdmag[:, :, :], mask=smask[:, :, :],
                                          data=mx[:, :, :])
                nc.vector.tensor_tensor(out=mx[:, :, :], in0=ubuf[:, :, 1:513],
                                        in1=dbuf[:, :, 1:513], op=OP.max)  # M2
                nc.vector.scalar_tensor_tensor(
                    out=cmask[:, :, :], in0=q1[:, :, :], scalar=float(T2SQ),
                    in1=q2[:, :, :], op0=OP.mult, op1=OP.is_lt)            # c2
                nc.vector.copy_predicated(out=mdmag[:, :, :], mask=cmask[:, :, :],
                                          data=mx[:, :, :])
                nc.vector.tensor_tensor(out=mx[:, :, :], in0=msqp[:, :, 2:514],
                                        in1=msqp[:, :, 0:512], op=OP.max)  # M0
                nc.vector.scalar_tensor_tensor(
                    out=cmask[:, :, :], in0=q1[:, :, :], scalar=float(T1SQ),
                    in1=q2[:, :, :], op0=OP.mult, op1=OP.is_gt)            # c0
                nc.vector.copy_predicated(out=mdmag[:, :, :], mask=cmask[:, :, :],
                                          data=mx[:, :, :])
                # keep = msq > Md
                nc.vector.tensor_tensor(out=cmask[:, :, :], in0=msqp[:, :, 1:513],
                                        in1=mdmag[:, :, :], op=OP.is_gt)
                # mag = sqrt(msq + 1e-6)  (overwrites Md)
                sqrt_i = nc.scalar.activation(mdmag[:, :, :], msqp[:, :, 1:513],
                                              AF.Sqrt, bias=b_eps[:, :])
                nc.vector.tensor_tensor(out=nm[:, :, :], in0=cmask[:, :, :],
                                        in1=mdmag[:, :, :], op=OP.mult)
                return sqrt_i

            def phase_b(img, nm):
                par = img % 2
                he = h8e[par]
                ho = h8o[par]
                p2 = p2b[par]
                sig1 = nc.scalar.activation(q1[:, :, :], nm[:, :, :], AF.Sigmoid,
                                            bias=b_hi[:, :], scale=10.0)
                sig2 = nc.scalar.activation(q2[:, :, :], nm[:, :, :], AF.Sigmoid,
                                            bias=b_lo[:, :], scale=10.0)
                # hi8 = (nm * SC) * sig1 (f32 math, RNE to u8), split even/odd
                # via strided DVE reads so the store DMAs stay contiguous
                nc.vector.scalar_tensor_tensor(
                    out=he[:, :, :], in0=nm[:, :, 0:W:2], scalar=float(SC),
                    in1=q1[:, :, 0:W:2], op0=OP.mult, op1=OP.mult)
                nc.vector.scalar_tensor_tensor(
                    out=ho[:, :, :], in0=nm[:, :, 1:W:2], scalar=float(SC),
                    in1=q1[:, :, 1:W:2], op0=OP.mult, op1=OP.mult)
                # d4 = (nm * SCD) * (sig2 - sig1), 4-bit; lo = hi + d4/SCD on host
                nc.gpsimd.tensor_tensor(out=sB[:, :, 0:512], in0=q2[:, :, :],
                                        in1=q1[:, :, :], op=OP.subtract)
                nc.vector.scalar_tensor_tensor(
                    out=dscr2[:, :, :], in0=nm[:, :, 0:W:2], scalar=float(SCD),
                    in1=sB[:, :, 0:W:2], op0=OP.mult, op1=OP.mult)
                nc.vector.scalar_tensor_tensor(
                    out=dscr3[:, :, :], in0=nm[:, :, 1:W:2], scalar=float(SCD),
                    in1=sB[:, :, 1:W:2], op0=OP.mult, op1=OP.mult)
                # p2 = (d4_even << 4) | d4_odd
                nc.vector.tensor_single_scalar(
                    out=dscr2[:, :, :], in_=dscr2[:, :, :], scalar=4,
                    op=OP.logical_shift_left)
                nc.vector.tensor_tensor(out=p2[:, :, :], in0=dscr2[:, :, :],
                                        in1=dscr3[:, :, :], op=OP.bitwise_or)
                # store: per plane, chunks 0..3 then chunk 4
                nc.sync.dma_start(
                    yp[img, 0, 0:496, :].rearrange("(t p) w -> p t w", p=TR),
                    he[0:124, 0:4, :])
                nc.sync.dma_start(yp[img, 0, 496:512, :], he[0:16, 4, :])
                nc.sync.dma_start(
                    yp[img, 1, 0:496, :].rearrange("(t p) w -> p t w", p=TR),
                    ho[0:124, 0:4, :])
                nc.sync.dma_start(yp[img, 1, 496:512, :], ho[0:16, 4, :])
                nc.sync.dma_start(
                    yp[img, 2, 0:496, :].rearrange("(t p) w -> p t w", p=TR),
                    p2[0:124, 0:4, :])
                nc.sync.dma_start(yp[img, 2, 496:512, :], p2[0:16, 4, :])
                return sig1, sig2

            # pairs of images share one sqrt->sigmoid table transition; deps
            # force the ACT engine to batch sqrts before sigmoids per pair so
            # walrus inserts only 4 table loads total
            prev_sig = None
            for pair in range(B_PER_CORE // 2):
                sq0 = phase_a(2 * pair, nms[0])
                if prev_sig is not None:
                    add_dep_helper(sq0.ins, prev_sig.ins, sync=False,
                                   reason="act table phase order")
                sq1 = phase_a(2 * pair + 1, nms[1])
                s0a, s0b = phase_b(2 * pair, nms[0])
                add_dep_helper(s0a.ins, sq1.ins, sync=False,
                               reason="act table phase order")
                s1a, s1b = phase_b(2 * pair + 1, nms[1])
                prev_sig = s1b

    nc.compile()
    return nc


_NC_CACHE = None


def _get_nc():
    global _NC_CACHE
    if _NC_CACHE is None:
        _NC_CACHE = build_bass()
    return _NC_CACHE


_POOL = ThreadPoolExecutor(max_workers=8)


def _weight_rows():
    # SBUF layout [partition k, (i m)] per direction, flattened f32 -> u16
    wx = np.stack(WGX_NP).transpose(1, 0, 2).reshape(P, 3 * P)
    wy = np.stack(WGY_NP).transpose(1, 0, 2).reshape(P, 3 * P)
    wb = np.concatenate([wx.reshape(-1), wy.reshape(-1)]).astype(np.float32)
    return wb.view(np.uint16).reshape(B_PER_CORE, 96, W)


# persistent input buffer: weight rows are constant, image rows rewritten
_XQ = np.empty((N_CORES * B_PER_CORE, 1, HX, W), np.uint16)
_XQ[:, 0, H:HX, :] = np.tile(_weight_rows(), (N_CORES, 1, 1)).reshape(
    N_CORES * B_PER_CORE, 96, W)


def _quantize_input(x):
    # channel-sum -> uint16 gq = round(sum * XS / 3); per-core in parallel
    def work(c):
        s = slice(c * B_PER_CORE, (c + 1) * B_PER_CORE)
        t = x[s, 0] + x[s, 1]
        np.add(t, x[s, 2], out=t)
        np.multiply(t, XS / 3.0, out=t)
        np.add(t, 0.5, out=t)
        _XQ[s, 0, 0:H, :] = t.astype(np.uint16)

    list(_POOL.map(work, range(N_CORES)))
    return _XQ


def _dequantize(parts):
    # yp u8 [B,3,H,W/2] per core -> (hi, lo) f32
    # planes: hi8 evens, hi8 odds, (d4_even << 4) | d4_odd
    hi = np.empty((N_CORES * B_PER_CORE, 1, H, W), np.float32)
    lo = np.empty((N_CORES * B_PER_CORE, 1, H, W), np.float32)

    def work(c):
        s = slice(c * B_PER_CORE, (c + 1) * B_PER_CORE)
        p = parts[c]
        hv = hi[s, 0]
        lv = lo[s, 0]
        np.multiply(p[:, 0].astype(np.float32), 1.0 / SC, out=hv[:, :, 0::2])
        np.multiply(p[:, 1].astype(np.float32), 1.0 / SC, out=hv[:, :, 1::2])
        d = (p[:, 2] >> 4).astype(np.float32)
        np.multiply(d, 1.0 / SCD, out=d)
        np.add(d, hv[:, :, 0::2], out=lv[:, :, 0::2])
        d = (p[:, 2] & 15).astype(np.float32)
        np.multiply(d, 1.0 / SCD, out=d)
        np.add(d, hv[:, :, 1::2], out=lv[:, :, 1::2])

    list(_POOL.map(work, range(N_CORES)))
    return hi, lo


_TIMED = bool(int(os.environ.get("KTIME", "0")))


def kernel(x: np.ndarray):
    from concourse import bass_utils

    t0 = time.perf_counter()
    x = np.asarray(x)
    assert x.shape == (32, 3, H, W), x.shape
    if x.dtype != np.float32:
        x = x.astype(np.float32)
    nc = _get_nc()
    xq = _quantize_input(x)
    in_maps = []
    for c in range(N_CORES):
        in_maps.append({
            "x": xq[c * B_PER_CORE:(c + 1) * B_PER_CORE],
        })
    t1 = time.perf_counter()
    res = bass_utils.run_bass_kernel_spmd(nc, in_maps,
                                          core_ids=list(range(N_CORES)))
    t2 = time.perf_counter()
    hi, lo = _dequantize([r["yp"] for r in res.results])
    t3 = time.perf_counter()
    if _TIMED:
        print(f"[ktime] quant {1e3*(t1-t0):.0f} ms | run {1e3*(t2-t1):.0f} ms"
              f" | dequant {1e3*(t3-t2):.0f} ms")
    return hi, lo
